# revision 6
# baseline (speedup 1.0000x reference)
"""ChessBoardAttention Trainium2 kernel (v2: S^T orientation, fp8 DoubleRow).

Math (per chessboard window of the input):
  x: [B=2, C=128, H=256, W=256] f32.  WS=8 chessboard phases.
  window (b, ph, pw) owns tokens (h, w) with h%8==ph, w%8==pw -> N=1024 tokens.
  q = Wq x + bq; k = Wk x (+bk; bk is softmax-shift-invariant and dropped)
  S[n, m] = q[n].k[m] = x[:,n]^T (Wq^T Wk) x[:,m] + (Wk^T bq).x[:,m]
  out = softmax_m(S) v ; y = gamma*out + x

Key ideas vs the v1 kernel (which was tensor/act/DMA-transpose bound):
  * Fused scores: host precomputes A = Wk^T Wq and u = Wk^T bq. On device
    kk = A^T x (one matmul) and S^T[m, n] = kk[:,m].x[:,n] + u.x[:,m] --
    no separate q/k projections, no q/k bias casts.
  * S^T (scores transposed, m on partitions) means attn^T is produced
    directly: NO DMA XBAR transposes anywhere.  exp(S^T) goes straight
    to fp8e4 in SBUF; AV and the softmax row-sum Z are fp8 DoubleRow
    matmuls (2 contraction tiles/pass = 2x PE throughput).
  * Z[n] = sum_m exp() via an all-ones fp8 stationary, replicated to all
    128 partitions so 1/Z is a plain DVE reciprocal (no broadcast op).
  * Normalization is deferred past AV: y = (po*gamma)*zb + (gv + x),
    two elementwise ops (DVE stt + Pool stt). gv = gamma*bv.
  * v^T is computed directly as vt[m, c] = (x chunk)^T Wv^T per 128-token
    chunk (stationary = x chunk), cast to fp8 -- no XBAR, no bf16 stage.
  * exp bias: cbias[m] = u.x[:,m] column from 8 tiny matmuls; applied as
    the ACT activation bias (free) / folded into Schraudolph constants.
  * exp split: 6 chunks on ACT (exact exp), 2 on DVE via Schraudolph
    (one tensor_scalar producing fp8e4 bit patterns).
  * PSUM (8 banks): S 2x[128,1024] (4) + rotating kv bank (1) + po (2)
    + Z (1).  Z halves serialize through one bank.
  * Per-window x loads/stores on the SP HW queue, prefetched 3 deep.

Sharding: 16 row-groups (b, ph), 2 per core, as v1.  xs[g, c, pw, t] is
host-permuted window-major; all compute for a window on one core.
"""

import sys

if "/opt/trn_rl_repo" not in sys.path:
    sys.path.insert(0, "/opt/trn_rl_repo")

from contextlib import ExitStack

import numpy as np

import concourse.bacc as bacc
import concourse.bass as bass
import concourse.mybir as mybir
from concourse import bass_utils
from concourse.tile import TileContext

B, C, H, W = 2, 128, 256, 256
WS = 8
NH, NW = H // WS, W // WS  # 32, 32
N = NH * NW  # 1024 tokens per window
D = C // 4  # 32 q/k channels
NCORES = 8
PAIRS = 2  # (b, ph) row-groups per core
NCH = N // 128  # 8 chunks of 128 tokens
NWIN = PAIRS * WS  # 16 windows per core
F32 = mybir.dt.float32
BF16 = mybir.dt.bfloat16
FP8 = mybir.dt.float8e4
I8 = mybir.dt.int8
AF = mybir.ActivationFunctionType
ALU = mybir.AluOpType
DR = mybir.MatmulPerfMode.DoubleRow

# chunks 0..NEXP_ACT-1 exp on ACT engine; the rest via DVE Schraudolph
NEXP_ACT = 6
# Schraudolph constants for fp8e4m3 bit patterns: bits ~= A*s + B
SCH_A = 8.0 / np.log(2.0)  # 11.5416
SCH_B = 56.0 - 0.344  # mean-corrected exponent bias

TRACE = False
LAST = {}
_CACHE = {}


def _emit(nc: bass.Bass):
    # xs is HOST-PERMUTED window-major: xs[g, c, pw, t] = x[b, c, (t//32)*8+ph, (t%32)*8+pw]
    xs = nc.dram_tensor("xs", [PAIRS, C, WS, N], F32, kind="ExternalInput").ap()
    ah = nc.dram_tensor("ah", [C, C], BF16, kind="ExternalInput").ap()  # Wk^T Wq
    wv = nc.dram_tensor("wv", [C, C], BF16, kind="ExternalInput").ap()  # Wv^T
    uh = nc.dram_tensor("uh", [C, 1], BF16, kind="ExternalInput").ap()  # Wk^T bq
    gam = nc.dram_tensor("gam", [C, 1], F32, kind="ExternalInput").ap()  # gamma
    ys = nc.dram_tensor("ys", [PAIRS, C, WS, N], F32, kind="ExternalOutput").ap()

    with ExitStack() as ctx:
        tc = ctx.enter_context(TileContext(nc))
        consts = ctx.enter_context(tc.tile_pool(name="consts", bufs=1))
        # bufs=5: load_x(w+3) reuses x(w-2)'s slot, whose store_y(w-2) was
        # emitted the PREVIOUS iteration on the same SP queue. With 4 bufs the
        # reused slot's store sits BEHIND the load in-queue -> deadlock.
        xpool = ctx.enter_context(tc.tile_pool(name="xpool", bufs=5))
        xbpool = ctx.enter_context(tc.tile_pool(name="xbpool", bufs=3))
        kkpool = ctx.enter_context(tc.tile_pool(name="kkpool", bufs=2))
        vtpool = ctx.enter_context(tc.tile_pool(name="vtpool", bufs=2))
        cbpool = ctx.enter_context(tc.tile_pool(name="cbpool", bufs=2))
        etpool = ctx.enter_context(tc.tile_pool(name="etpool", bufs=2))
        zbpool = ctx.enter_context(tc.tile_pool(name="zbpool", bufs=2))
        tmppool = ctx.enter_context(tc.tile_pool(name="tmppool", bufs=2))
        # PSUM: 8 banks total
        ps_s = ctx.enter_context(tc.tile_pool(name="ps_s", bufs=2, space="PSUM"))
        ps_kv = ctx.enter_context(tc.tile_pool(name="ps_kv", bufs=1, space="PSUM"))
        ps_o = ctx.enter_context(tc.tile_pool(name="ps_o", bufs=1, space="PSUM"))
        ps_z = ctx.enter_context(tc.tile_pool(name="ps_z", bufs=1, space="PSUM"))

        ah_sb = consts.tile([C, C], BF16)
        nc.sync.dma_start(out=ah_sb, in_=ah)
        wv_sb = consts.tile([C, C], BF16)
        nc.sync.dma_start(out=wv_sb, in_=wv)
        uh_sb = consts.tile([C, 1], BF16)
        nc.sync.dma_start(out=uh_sb, in_=uh)
        gam_sb = consts.tile([C, 1], F32)
        nc.sync.dma_start(out=gam_sb, in_=gam)
        ones_sb = consts.tile([C, 2, C], FP8)
        nc.vector.memset(ones_sb, 1.0)

        # Touch consts on the engines that use them as TensorScalarPtr args
        # (walrus allows ONE sync wait on those; strip the const-DMA wait).
        scr_v = consts.tile([C, 4], F32)
        for i, t in enumerate([gam_sb, ah_sb, uh_sb, wv_sb]):
            nc.vector.tensor_copy(out=scr_v[: t.shape[0], i : i + 1], in_=t[:, 0:1])


        # PE warm-up: keep the array busy while the first x slab loads so the
        # clock ramps; results are never read.
        warm = ps_s.tile([C, N], F32, tag="st", name="warm")
        for i in range(24):
            nc.tensor.matmul(
                warm[:, 0:128], wv_sb, wv_sb, skip_group_check=True
            )

        xt, xbt, kkt, vtt, cbt, ett = {}, {}, {}, {}, {}, {}
        pot, zht, zbt = {}, {}, {}

        def load_x(w):
            g, pw = divmod(w, WS)
            xt[w] = xpool.tile([C, N], F32, tag="x", name=f"x{w}")
            nc.sync.dma_start(out=xt[w], in_=xs[g, :, pw, :])

        def cast_xb(w):
            xbt[w] = xbpool.tile([C, N], BF16, tag="xb", name=f"xb{w}")
            nc.gpsimd.tensor_copy(out=xbt[w], in_=xt[w])

        def kk_half(w, h):
            if h == 0:
                kkt[w] = kkpool.tile([C, N], BF16, tag="kk", name=f"kk{w}")
            kv = ps_kv.tile([C, 512], F32, tag="kv", name=f"kvk{w}_{h}")
            nc.tensor.matmul(kv, ah_sb, xbt[w][:, bass.ts(h, 512)])
            nc.vector.tensor_copy(out=kkt[w][:, bass.ts(h, 512)], in_=kv)

        def vt_half(w, h):
            if h == 0:
                vtt[w] = vtpool.tile([C, NCH, C], FP8, tag="vt", name=f"vt{w}")
            kv = ps_kv.tile([C, 4, C], F32, tag="kv", name=f"kvv{w}_{h}")
            for j in range(4):
                mc = 4 * h + j
                nc.tensor.matmul(
                    kv[:, j, :],
                    xbt[w][:, bass.ts(mc, 128)],
                    wv_sb,
                    start=(j == 0),
                    stop=(j == 3),
                    skip_group_check=True,
                )
            nc.vector.tensor_copy(
                out=vtt[w][:, 4 * h : 4 * h + 4, :], in_=kv
            )

        def cps_mms(w):
            # cbias[m] = u . x[:, m] per 128-token chunk, + Schraudolph form
            kv = ps_kv.tile([C, NCH], F32, tag="kv", name=f"kvc{w}")
            for mc in range(NCH):
                nc.tensor.matmul(
                    kv[:, mc : mc + 1],
                    xbt[w][:, bass.ts(mc, 128)],
                    uh_sb,
                    start=(mc == 0),
                    stop=(mc == NCH - 1),
                    skip_group_check=True,
                )
            cbt[w] = cbpool.tile([C, 2, NCH], F32, tag="cb", name=f"cb{w}")
            nc.vector.tensor_copy(out=cbt[w][:, 0, :], in_=kv)
            nc.vector.tensor_scalar(
                out=cbt[w][:, 1, :],
                in0=kv,
                scalar1=float(SCH_A),
                scalar2=float(SCH_B),
                op0=ALU.mult,
                op1=ALU.add,
            )

        def s_chunk(w, mc):
            st = ps_s.tile([C, N], F32, tag="st", name=f"st{w}_{mc}")
            for h in range(2):
                nc.tensor.matmul(
                    st[:, bass.ts(h, 512)],
                    kkt[w][:, bass.ts(mc, 128)],
                    xbt[w][:, bass.ts(h, 512)],
                )
            if mc < NEXP_ACT:
                nc.scalar.activation(
                    out=ett[w][:, mc, :],
                    in_=st,
                    func=AF.Exp,
                    bias=cbt[w][:, 0, mc : mc + 1],
                )
            else:
                nc.vector.tensor_scalar(
                    out=ett[w][:, mc, :].bitcast(I8),
                    in0=st,
                    scalar1=float(SCH_A),
                    scalar2=cbt[w][:, 1, mc : mc + 1],
                    op0=ALU.mult,
                    op1=ALU.add,
                )

        def avz_pair(w, p):
            if p == 0:
                pot[w] = ps_o.tile([C, N], F32, tag="po", name=f"po{w}")
                zht[(w, 0)] = ps_z.tile([C, 512], F32, tag="zz", name=f"z{w}h0")
            et2 = ett[w][:, 2 * p : 2 * p + 2, :]
            for h in range(2):
                nc.tensor.matmul(
                    pot[w][:, bass.ts(h, 512)],
                    vtt[w][:, 2 * p : 2 * p + 2, :],
                    et2[:, :, bass.ts(h, 512)],
                    start=(p == 0),
                    stop=(p == 3),
                    perf_mode=DR,
                    skip_group_check=True,
                )
            nc.tensor.matmul(
                zht[(w, 0)],
                ones_sb,
                et2[:, :, 0:512],
                start=(p == 0),
                stop=(p == 3),
                perf_mode=DR,
                skip_group_check=True,
            )

        def recip_h0(w):
            zbt[w] = zbpool.tile([C, N], F32, tag="zb", name=f"zb{w}")
            nc.vector.reciprocal(out=zbt[w][:, 0:512], in_=zht[(w, 0)])

        def z_h1(w):
            zht[(w, 1)] = ps_z.tile([C, 512], F32, tag="zz", name=f"z{w}h1")
            for p in range(4):
                nc.tensor.matmul(
                    zht[(w, 1)],
                    ones_sb,
                    ett[w][:, 2 * p : 2 * p + 2, 512:1024],
                    start=(p == 0),
                    stop=(p == 3),
                    perf_mode=DR,
                    skip_group_check=True,
                )

        def epilogue(w):
            nc.vector.reciprocal(out=zbt[w][:, 512:1024], in_=zht[(w, 1)])
            tmp = tmppool.tile([C, N], F32, tag="tmp", name=f"tmp{w}")
            nc.vector.scalar_tensor_tensor(
                out=tmp,
                in0=pot[w],
                scalar=gam_sb,
                in1=zbt[w],
                op0=ALU.mult,
                op1=ALU.mult,
            )
            nc.gpsimd.tensor_tensor(
                out=xt[w], in0=tmp, in1=xt[w], op=ALU.add
            )
            g, pw = divmod(w, WS)
            nc.sync.dma_start(out=ys[g, :, pw, :], in_=xt[w])

        # -------- schedule --------
        load_x(0)
        load_x(1)
        load_x(2)
        cast_xb(0)
        cast_xb(1)
        kk_half(0, 0)
        kk_half(0, 1)
        vt_half(0, 0)
        vt_half(0, 1)
        cps_mms(0)

        for w in range(NWIN):
            ett[w] = etpool.tile([C, NCH, N], FP8, tag="et", name=f"et{w}")
            if w + 3 < NWIN:
                load_x(w + 3)
            if w + 2 < NWIN:
                cast_xb(w + 2)
            s_chunk(w, 0)
            s_chunk(w, 1)
            if w >= 1:
                z_h1(w - 1)
            if w + 1 < NWIN:
                kk_half(w + 1, 0)
            s_chunk(w, 2)
            if w >= 1:
                epilogue(w - 1)
            if w + 1 < NWIN:
                kk_half(w + 1, 1)
            avz_pair(w, 0)
            s_chunk(w, 3)
            if w + 1 < NWIN:
                vt_half(w + 1, 0)
            s_chunk(w, 4)
            avz_pair(w, 1)
            if w + 1 < NWIN:
                vt_half(w + 1, 1)
            s_chunk(w, 5)
            avz_pair(w, 2)
            if w + 1 < NWIN:
                cps_mms(w + 1)
            s_chunk(w, 6)
            s_chunk(w, 7)
            avz_pair(w, 3)
            recip_h0(w)
            # free stale per-window refs
            for dd in (xt, xbt, kkt, vtt, cbt, ett, pot, zbt):
                dd.pop(w - 2, None)

        z_h1(NWIN - 1)
        epilogue(NWIN - 1)
    return nc


def _get_nc():
    if "nc" not in _CACHE:
        nc = bacc.Bacc(
            "TRN2",
            target_bir_lowering=False,
            debug=False,
            enable_asserts=False,
            num_devices=NCORES,
        )
        _emit(nc)
        nc.finalize()
        _CACHE["nc"] = nc
    return _CACHE["nc"]


def _shard_inputs(x, Wq, bq, Wk, bk, Wv, bv, gamma):
    import ml_dtypes

    bf = ml_dtypes.bfloat16
    x = np.ascontiguousarray(np.asarray(x, np.float32))
    Wq = np.asarray(Wq, np.float32)
    Wk = np.asarray(Wk, np.float32)
    Wv = np.asarray(Wv, np.float32)
    bq = np.asarray(bq, np.float32)
    g = float(np.asarray(gamma, np.float32).reshape(-1)[0])
    ah_h = np.ascontiguousarray(Wk.T @ Wq).astype(bf)  # [C, C]
    wv_h = np.ascontiguousarray(Wv.T).astype(bf)  # [C, C]
    # beta folds the value bias + residual constant into the input:
    # y = gamma*(attn@(Wv xg)) / Z + xg  with xg = x + beta, where
    # (I + gamma*Wv) beta = gamma*bv.  Score bias shifts to u' = Wk^T(bq+Wq beta).
    bv = np.asarray(bv, np.float64)
    beta = np.linalg.solve(
        np.eye(C, dtype=np.float64) + g * Wv.astype(np.float64), g * bv
    ).astype(np.float32)
    uh_h = np.ascontiguousarray((Wk.T @ (bq + Wq @ beta)).reshape(C, 1)).astype(bf)
    gam_h = np.full((C, 1), g, np.float32)
    x = x + beta.reshape(1, C, 1, 1)
    # window-major permute: x6[b, c, i, ph, j, pw] -> slab[c, pw, i*32+j]
    x6 = x.reshape(B, C, NH, WS, NW, WS)
    in_maps = []
    for core in range(NCORES):
        slabs = np.stack(
            [
                np.ascontiguousarray(
                    x6[(PAIRS * core + j) // WS, :, :, (PAIRS * core + j) % WS, :, :]
                    .transpose(0, 3, 1, 2)  # [c, pw, i, j]
                    .reshape(C, WS, N)
                )
                for j in range(PAIRS)
            ]
        )
        in_maps.append(
            dict(xs=slabs, ah=ah_h, wv=wv_h, uh=uh_h, gam=gam_h)
        )
    return in_maps


def kernel(x, Wq, bq, Wk, bk, Wv, bv, gamma):
    nc = _get_nc()
    in_maps = _shard_inputs(x, Wq, bq, Wk, bk, Wv, bv, gamma)
    res = bass_utils.run_bass_kernel_spmd(
        nc, in_maps, core_ids=list(range(NCORES)), trace=TRACE
    )
    LAST["exec_time_ns"] = res.exec_time_ns
    LAST["results"] = res
    y = np.empty((B, C, H, W), np.float32)
    y6 = y.reshape(B, C, NH, WS, NW, WS)
    for core in range(NCORES):
        out = res.results[core]["ys"]  # [PAIRS, C, WS, N]
        for j in range(PAIRS):
            p = PAIRS * core + j
            # [c, pw, i, j] -> [c, i, j, pw]
            y6[p // WS, :, :, p % WS, :, :] = (
                out[j].reshape(C, WS, NH, NW).transpose(0, 2, 3, 1)
            )
    return y


# revision 7
# speedup vs baseline: 1.7727x; 1.7727x over previous
"""ChessBoardAttention Trainium2 kernel (v2: S^T orientation, fp8 DoubleRow).

Math (per chessboard window of the input):
  x: [B=2, C=128, H=256, W=256] f32.  WS=8 chessboard phases.
  window (b, ph, pw) owns tokens (h, w) with h%8==ph, w%8==pw -> N=1024 tokens.
  q = Wq x + bq; k = Wk x (+bk; bk is softmax-shift-invariant and dropped)
  S[n, m] = q[n].k[m] = x[:,n]^T (Wq^T Wk) x[:,m] + (Wk^T bq).x[:,m]
  out = softmax_m(S) v ; y = gamma*out + x

Key ideas vs the v1 kernel (which was tensor/act/DMA-transpose bound):
  * Fused scores: host precomputes A = Wk^T Wq and u = Wk^T bq. On device
    kk = A^T x (one matmul) and S^T[m, n] = kk[:,m].x[:,n] + u.x[:,m] --
    no separate q/k projections, no q/k bias casts.
  * S^T (scores transposed, m on partitions) means attn^T is produced
    directly: NO DMA XBAR transposes anywhere.  exp(S^T) goes straight
    to fp8e4 in SBUF; AV and the softmax row-sum Z are fp8 DoubleRow
    matmuls (2 contraction tiles/pass = 2x PE throughput).
  * Z[n] = sum_m exp() via an all-ones fp8 stationary, replicated to all
    128 partitions so 1/Z is a plain DVE reciprocal (no broadcast op).
  * Normalization is deferred past AV: y = (po*gamma)*zb + (gv + x),
    two elementwise ops (DVE stt + Pool stt). gv = gamma*bv.
  * v^T is computed directly as vt[m, c] = (x chunk)^T Wv^T per 128-token
    chunk (stationary = x chunk), cast to fp8 -- no XBAR, no bf16 stage.
  * exp bias: cbias[m] = u.x[:,m] column from 8 tiny matmuls; applied as
    the ACT activation bias (free) / folded into Schraudolph constants.
  * exp split: 6 chunks on ACT (exact exp), 2 on DVE via Schraudolph
    (one tensor_scalar producing fp8e4 bit patterns).
  * PSUM (8 banks): S 2x[128,1024] (4) + rotating kv bank (1) + po (2)
    + Z (1).  Z halves serialize through one bank.
  * Per-window x loads/stores on the SP HW queue, prefetched 3 deep.

Sharding: 16 row-groups (b, ph), 2 per core, as v1.  xs[g, c, pw, t] is
host-permuted window-major; all compute for a window on one core.
"""

import sys

if "/opt/trn_rl_repo" not in sys.path:
    sys.path.insert(0, "/opt/trn_rl_repo")

from contextlib import ExitStack

import numpy as np

import concourse.bacc as bacc
import concourse.bass as bass
import concourse.mybir as mybir
from concourse import bass_utils
from concourse.tile import TileContext

B, C, H, W = 2, 128, 256, 256
WS = 8
NH, NW = H // WS, W // WS  # 32, 32
N = NH * NW  # 1024 tokens per window
D = C // 4  # 32 q/k channels
NCORES = 8
PAIRS = 2  # (b, ph) row-groups per core
NCH = N // 128  # 8 chunks of 128 tokens
NWIN = PAIRS * WS  # 16 windows per core
F32 = mybir.dt.float32
BF16 = mybir.dt.bfloat16
FP8 = mybir.dt.float8e4
I8 = mybir.dt.int8
I32 = mybir.dt.int32
AF = mybir.ActivationFunctionType
ALU = mybir.AluOpType
DR = mybir.MatmulPerfMode.DoubleRow

# chunks 0..NEXP_ACT-1 exp on ACT engine; the rest via DVE Schraudolph
NEXP_ACT = 4
# bits(1/Z) ~= RECIP_K - bits(Z): magic-constant reciprocal (max ~5% rel err,
# scaled by gamma*attn so ~1e-3 of output scale)
RECIP_K = 0x7EF30000
# Schraudolph constants for fp8e4m3 bit patterns: bits ~= A*s + B
SCH_A = 8.0 / np.log(2.0)  # 11.5416
SCH_B = 56.0 - 0.344  # mean-corrected exponent bias

TRACE = False
LAST = {}
_CACHE = {}


def _emit(nc: bass.Bass):
    # xs is HOST-PERMUTED window-major: xs[g, c, pw, t] = x[b, c, (t//32)*8+ph, (t%32)*8+pw]
    xs = nc.dram_tensor("xs", [PAIRS, C, WS, N], F32, kind="ExternalInput").ap()
    ah = nc.dram_tensor("ah", [C, C], BF16, kind="ExternalInput").ap()  # Wk^T Wq
    wv = nc.dram_tensor("wv", [C, C], BF16, kind="ExternalInput").ap()  # Wv^T
    uh = nc.dram_tensor("uh", [C, 1], BF16, kind="ExternalInput").ap()  # Wk^T bq
    gam = nc.dram_tensor("gam", [C, 1], F32, kind="ExternalInput").ap()  # gamma
    ys = nc.dram_tensor("ys", [PAIRS, C, WS, N], F32, kind="ExternalOutput").ap()

    with ExitStack() as ctx:
        tc = ctx.enter_context(TileContext(nc))
        consts = ctx.enter_context(tc.tile_pool(name="consts", bufs=1))
        # bufs=5: load_x(w+3) reuses x(w-2)'s slot, whose store_y(w-2) was
        # emitted the PREVIOUS iteration on the same SP queue. With 4 bufs the
        # reused slot's store sits BEHIND the load in-queue -> deadlock.
        xpool = ctx.enter_context(tc.tile_pool(name="xpool", bufs=5))
        xbpool = ctx.enter_context(tc.tile_pool(name="xbpool", bufs=3))
        kkpool = ctx.enter_context(tc.tile_pool(name="kkpool", bufs=2))
        vtpool = ctx.enter_context(tc.tile_pool(name="vtpool", bufs=2))
        cbpool = ctx.enter_context(tc.tile_pool(name="cbpool", bufs=2))
        etpool = ctx.enter_context(tc.tile_pool(name="etpool", bufs=2))
        zbpool = ctx.enter_context(tc.tile_pool(name="zbpool", bufs=2))
        tmppool = ctx.enter_context(tc.tile_pool(name="tmppool", bufs=2))
        # PSUM: 8 banks total
        ps_s = ctx.enter_context(tc.tile_pool(name="ps_s", bufs=2, space="PSUM"))
        ps_kv = ctx.enter_context(tc.tile_pool(name="ps_kv", bufs=1, space="PSUM"))
        ps_o = ctx.enter_context(tc.tile_pool(name="ps_o", bufs=1, space="PSUM"))
        ps_z = ctx.enter_context(tc.tile_pool(name="ps_z", bufs=1, space="PSUM"))

        ah_sb = consts.tile([C, C], BF16)
        nc.sync.dma_start(out=ah_sb, in_=ah)
        wv_sb = consts.tile([C, C], BF16)
        nc.sync.dma_start(out=wv_sb, in_=wv)
        uh_sb = consts.tile([C, 1], BF16)
        nc.sync.dma_start(out=uh_sb, in_=uh)
        gam_sb = consts.tile([C, 1], F32)
        nc.sync.dma_start(out=gam_sb, in_=gam)
        ones_sb = consts.tile([C, 2, C], FP8)
        nc.vector.memset(ones_sb, 1.0)

        # Touch consts on the engines that use them as TensorScalarPtr args
        # (walrus allows ONE sync wait on those; strip the const-DMA wait).
        scr_v = consts.tile([C, 4], F32)
        for i, t in enumerate([gam_sb, ah_sb, uh_sb, wv_sb]):
            nc.vector.tensor_copy(out=scr_v[: t.shape[0], i : i + 1], in_=t[:, 0:1])


        # PE warm-up: keep the array busy while the first x slab loads so the
        # clock ramps; results are never read.
        warm = ps_s.tile([C, N], F32, tag="st", name="warm")
        for i in range(24):
            nc.tensor.matmul(
                warm[:, 0:128], wv_sb, wv_sb, skip_group_check=True
            )

        xt, xbt, kkt, vtt, cbt, ett = {}, {}, {}, {}, {}, {}
        pot, zht, zbt = {}, {}, {}

        def load_x(w):
            g, pw = divmod(w, WS)
            xt[w] = xpool.tile([C, N], F32, tag="x", name=f"x{w}")
            nc.sync.dma_start(out=xt[w], in_=xs[g, :, pw, :])

        def cast_xb(w):
            g, pw = divmod(w, WS)
            xbt[w] = xbpool.tile([C, N], BF16, tag="xb", name=f"xb{w}")
            nc.gpsimd.dma_start(out=xbt[w], in_=xs[g, :, pw, :])

        def kk_half(w, h):
            if h == 0:
                kkt[w] = kkpool.tile([C, N], BF16, tag="kk", name=f"kk{w}")
            kv = ps_kv.tile([C, 512], F32, tag="kv", name=f"kvk{w}_{h}")
            nc.tensor.matmul(kv, ah_sb, xbt[w][:, bass.ts(h, 512)])
            nc.vector.tensor_copy(out=kkt[w][:, bass.ts(h, 512)], in_=kv)

        def vt_half(w, h):
            if h == 0:
                vtt[w] = vtpool.tile([C, NCH, C], FP8, tag="vt", name=f"vt{w}")
            kv = ps_kv.tile([C, 4, C], F32, tag="kv", name=f"kvv{w}_{h}")
            for j in range(4):
                mc = 4 * h + j
                nc.tensor.matmul(
                    kv[:, j, :],
                    xbt[w][:, bass.ts(mc, 128)],
                    wv_sb,
                    start=(j == 0),
                    stop=(j == 3),
                    skip_group_check=True,
                )
            nc.scalar.activation(
                out=vtt[w][:, 4 * h : 4 * h + 4, :], in_=kv, func=AF.Copy
            )

        def cps_mms(w):
            # cbias[m] = u . x[:, m] per 128-token chunk, + Schraudolph form
            kv = ps_kv.tile([C, NCH], F32, tag="kv", name=f"kvc{w}")
            for mc in range(NCH):
                nc.tensor.matmul(
                    kv[:, mc : mc + 1],
                    xbt[w][:, bass.ts(mc, 128)],
                    uh_sb,
                    start=(mc == 0),
                    stop=(mc == NCH - 1),
                    skip_group_check=True,
                )
            cbt[w] = cbpool.tile([C, 2, NCH], F32, tag="cb", name=f"cb{w}")
            nc.vector.tensor_copy(out=cbt[w][:, 0, :], in_=kv)
            nc.vector.tensor_scalar(
                out=cbt[w][:, 1, :],
                in0=kv,
                scalar1=float(SCH_A),
                scalar2=float(SCH_B),
                op0=ALU.mult,
                op1=ALU.add,
            )

        def s_chunk(w, mc):
            st = ps_s.tile([C, N], F32, tag="st", name=f"st{w}_{mc}")
            for h in range(2):
                nc.tensor.matmul(
                    st[:, bass.ts(h, 512)],
                    kkt[w][:, bass.ts(mc, 128)],
                    xbt[w][:, bass.ts(h, 512)],
                )
            if mc < NEXP_ACT:
                nc.scalar.activation(
                    out=ett[w][:, mc, :],
                    in_=st,
                    func=AF.Exp,
                    bias=cbt[w][:, 0, mc : mc + 1],
                )
            else:
                nc.vector.tensor_scalar(
                    out=ett[w][:, mc, :].bitcast(I8),
                    in0=st,
                    scalar1=float(SCH_A),
                    scalar2=cbt[w][:, 1, mc : mc + 1],
                    op0=ALU.mult,
                    op1=ALU.add,
                )

        def avz_pair(w, p):
            if p == 0:
                pot[w] = ps_o.tile([C, N], F32, tag="po", name=f"po{w}")
                zht[(w, 0)] = ps_z.tile([C, 512], F32, tag="zz", name=f"z{w}h0")
            et2 = ett[w][:, 2 * p : 2 * p + 2, :]
            for h in range(2):
                nc.tensor.matmul(
                    pot[w][:, bass.ts(h, 512)],
                    vtt[w][:, 2 * p : 2 * p + 2, :],
                    et2[:, :, bass.ts(h, 512)],
                    start=(p == 0),
                    stop=(p == 3),
                    perf_mode=DR,
                    skip_group_check=True,
                )
            nc.tensor.matmul(
                zht[(w, 0)],
                ones_sb,
                et2[:, :, 0:512],
                start=(p == 0),
                stop=(p == 3),
                perf_mode=DR,
                skip_group_check=True,
            )

        def recip_h0(w):
            zbt[w] = zbpool.tile([C, N], F32, tag="zb", name=f"zb{w}")
            nc.vector.tensor_scalar(
                out=zbt[w][:, 0:512].bitcast(I32),
                in0=zht[(w, 0)].bitcast(I32),
                scalar1=-1,
                scalar2=RECIP_K,
                op0=ALU.mult,
                op1=ALU.add,
            )

        def z_h1(w):
            zht[(w, 1)] = ps_z.tile([C, 512], F32, tag="zz", name=f"z{w}h1")
            for p in range(4):
                nc.tensor.matmul(
                    zht[(w, 1)],
                    ones_sb,
                    ett[w][:, 2 * p : 2 * p + 2, 512:1024],
                    start=(p == 0),
                    stop=(p == 3),
                    perf_mode=DR,
                    skip_group_check=True,
                )

        def epilogue(w):
            nc.vector.tensor_scalar(
                out=zbt[w][:, 512:1024].bitcast(I32),
                in0=zht[(w, 1)].bitcast(I32),
                scalar1=-1,
                scalar2=RECIP_K,
                op0=ALU.mult,
                op1=ALU.add,
            )
            tmp = tmppool.tile([C, N], F32, tag="tmp", name=f"tmp{w}")
            nc.vector.scalar_tensor_tensor(
                out=tmp,
                in0=pot[w],
                scalar=gam_sb,
                in1=zbt[w],
                op0=ALU.mult,
                op1=ALU.mult,
            )
            nc.gpsimd.tensor_tensor(
                out=xt[w], in0=tmp, in1=xt[w], op=ALU.add
            )
            g, pw = divmod(w, WS)
            nc.sync.dma_start(out=ys[g, :, pw, :], in_=xt[w])

        # -------- schedule --------
        load_x(0)
        load_x(1)
        load_x(2)
        cast_xb(0)
        cast_xb(1)
        kk_half(0, 0)
        kk_half(0, 1)
        vt_half(0, 0)
        vt_half(0, 1)
        cps_mms(0)

        for w in range(NWIN):
            ett[w] = etpool.tile([C, NCH, N], FP8, tag="et", name=f"et{w}")
            if w + 3 < NWIN:
                load_x(w + 3)
            if w + 2 < NWIN:
                cast_xb(w + 2)
            s_chunk(w, 0)
            s_chunk(w, 1)
            if w >= 1:
                z_h1(w - 1)
            if w + 1 < NWIN:
                kk_half(w + 1, 0)
            s_chunk(w, 2)
            if w >= 1:
                epilogue(w - 1)
            if w + 1 < NWIN:
                kk_half(w + 1, 1)
            avz_pair(w, 0)
            s_chunk(w, 3)
            if w + 1 < NWIN:
                vt_half(w + 1, 0)
            s_chunk(w, 4)
            avz_pair(w, 1)
            if w + 1 < NWIN:
                vt_half(w + 1, 1)
            s_chunk(w, 5)
            avz_pair(w, 2)
            if w + 1 < NWIN:
                cps_mms(w + 1)
            s_chunk(w, 6)
            s_chunk(w, 7)
            avz_pair(w, 3)
            recip_h0(w)
            # free stale per-window refs
            for dd in (xt, xbt, kkt, vtt, cbt, ett, pot, zbt):
                dd.pop(w - 2, None)

        z_h1(NWIN - 1)
        epilogue(NWIN - 1)
    return nc


def _get_nc():
    if "nc" not in _CACHE:
        nc = bacc.Bacc(
            "TRN2",
            target_bir_lowering=False,
            debug=False,
            enable_asserts=False,
            num_devices=NCORES,
        )
        _emit(nc)
        nc.finalize()
        _CACHE["nc"] = nc
    return _CACHE["nc"]


def _shard_inputs(x, Wq, bq, Wk, bk, Wv, bv, gamma):
    import ml_dtypes

    bf = ml_dtypes.bfloat16
    x = np.ascontiguousarray(np.asarray(x, np.float32))
    Wq = np.asarray(Wq, np.float32)
    Wk = np.asarray(Wk, np.float32)
    Wv = np.asarray(Wv, np.float32)
    bq = np.asarray(bq, np.float32)
    g = float(np.asarray(gamma, np.float32).reshape(-1)[0])
    ah_h = np.ascontiguousarray(Wk.T @ Wq).astype(bf)  # [C, C]
    wv_h = np.ascontiguousarray(Wv.T).astype(bf)  # [C, C]
    # beta folds the value bias + residual constant into the input:
    # y = gamma*(attn@(Wv xg)) / Z + xg  with xg = x + beta, where
    # (I + gamma*Wv) beta = gamma*bv.  Score bias shifts to u' = Wk^T(bq+Wq beta).
    bv = np.asarray(bv, np.float64)
    beta = np.linalg.solve(
        np.eye(C, dtype=np.float64) + g * Wv.astype(np.float64), g * bv
    ).astype(np.float32)
    uh_h = np.ascontiguousarray((Wk.T @ (bq + Wq @ beta)).reshape(C, 1)).astype(bf)
    gam_h = np.full((C, 1), g, np.float32)
    x = x + beta.reshape(1, C, 1, 1)
    # window-major permute: x6[b, c, i, ph, j, pw] -> slab[c, pw, i*32+j]
    x6 = x.reshape(B, C, NH, WS, NW, WS)
    in_maps = []
    for core in range(NCORES):
        slabs = np.stack(
            [
                np.ascontiguousarray(
                    x6[(PAIRS * core + j) // WS, :, :, (PAIRS * core + j) % WS, :, :]
                    .transpose(0, 3, 1, 2)  # [c, pw, i, j]
                    .reshape(C, WS, N)
                )
                for j in range(PAIRS)
            ]
        )
        in_maps.append(
            dict(xs=slabs, ah=ah_h, wv=wv_h, uh=uh_h, gam=gam_h)
        )
    return in_maps


def kernel(x, Wq, bq, Wk, bk, Wv, bv, gamma):
    nc = _get_nc()
    in_maps = _shard_inputs(x, Wq, bq, Wk, bk, Wv, bv, gamma)
    res = bass_utils.run_bass_kernel_spmd(
        nc, in_maps, core_ids=list(range(NCORES)), trace=TRACE
    )
    LAST["exec_time_ns"] = res.exec_time_ns
    LAST["results"] = res
    y = np.empty((B, C, H, W), np.float32)
    y6 = y.reshape(B, C, NH, WS, NW, WS)
    for core in range(NCORES):
        out = res.results[core]["ys"]  # [PAIRS, C, WS, N]
        for j in range(PAIRS):
            p = PAIRS * core + j
            # [c, pw, i, j] -> [c, i, j, pw]
            y6[p // WS, :, :, p % WS, :, :] = (
                out[j].reshape(C, WS, NH, NW).transpose(0, 2, 3, 1)
            )
    return y
